# revision 21
# baseline (speedup 1.0000x reference)
"""HQQ-compatible 4-bit quantized linear layer on 8 Trainium2 NeuronCores.

Problem: y = x @ W.T + bias where W = ((unpack4(W_q) - zero) * scale).reshape(8192, 8192)
  x: (64, 8192) f32; W_q: (32, 1048576) int32 (bytes, two nibbles packed);
  scale/zero: (1, 1048576) f32; bias: (8192,) f32.

Math per output element (OUT=IN=8192, GS=64, NG=2**20):
  W[o, i] = (Wu[gs, ng] - zero[ng]) * scale[ng],  gs = o // 128, ng = (o % 128)*8192 + i
  Wu[r, ng] = W_q[r, ng] >> 4 (r < 32) | W_q[r-32, ng] & 0xF (r >= 32).

Sharding (tensor-parallel over output features, by ng blocks):
  core m owns ng in [m*131072, (m+1)*131072)  <=>  (o % 128) in [m*16, m*16+16).

v3 design (measured-rate balanced; no GpSimd compute - its SBUF port is
shared with the DVE and slows Vector by ~1.5x):
  - hi nibbles: HWDGE u8 DMA -> ScalarE act-copy cast u8->bf16 (8 casts).
  - lo nibbles: SWDGE cast-DMA u8->bf16 (no engine time).
  - dequant: VectorE TT bf16 2x-mode, 16 ops of FD=4096 (~2.2us each).
  - matmuls col-tiled: hi -> PSUM[0:64) at tile_position (0,0), lo ->
    PSUM[64:128) at (0,64), streaming concurrently; zero-term psC and a
    K=1 ones x bias matmul fold bias straight into PSUM.
  - epilogue: y_half = psU_half + psC_bc via scalar_tensor_tensor, f32 out.
"""

import ml_dtypes
import numpy as np

OUT = 8192
IN = 8192
GS = 64
NG = OUT * IN // GS  # 1048576
B = 64
NCORES = 8
NGC = NG // NCORES   # 131072 groups per core
BB = 16              # width of the (o % 128) block per core
KT = IN // 128       # 64 in-tiles of 128
CHUNKS = [4, 4, 8, 8, 8, 8, 8, 8, 4, 4]  # k-tiles per compute chunk (sum 64)
PIECES = [4, 4, 8, 16, 16, 8, 8]         # k-tiles per DMA piece (sum 64)

_CACHE = {}


def _build_nc():
    import concourse.bacc as bacc
    import concourse.mybir as mybir
    import concourse.tile as tile
    from concourse.alu_op_type import AluOpType

    f16 = mybir.dt.bfloat16
    f32 = mybir.dt.float32
    u8 = mybir.dt.uint8

    nc = bacc.Bacc(None, target_bir_lowering=False, debug=False)

    xt_d = nc.dram_tensor("xt", [128, KT * B], f16, kind="ExternalInput")
    hi_d = nc.dram_tensor("hi", [128, KT * 512], u8, kind="ExternalInput")
    lo_d = nc.dram_tensor("lo", [128, KT * 512], u8, kind="ExternalInput")
    sc_d = nc.dram_tensor("sc", [128, KT * BB], f16, kind="ExternalInput")
    sz_d = nc.dram_tensor("sz", [128, KT * BB], f16, kind="ExternalInput")
    bs_d = nc.dram_tensor("bs", [1, 1024], f32, kind="ExternalInput")
    y_d = nc.dram_tensor("y", [B, 1024], f32, kind="ExternalOutput")

    with tile.TileContext(nc) as tc:
        with (
            tc.tile_pool(name="const", bufs=1) as cpool,
            tc.tile_pool(name="wq", bufs=1) as wqpool,
            tc.tile_pool(name="nib", bufs=2) as nibpool,
            tc.tile_pool(name="ws", bufs=3) as wspool,
            tc.tile_pool(name="psum", bufs=1, space="PSUM") as pspool,
            tc.tile_pool(name="outp", bufs=1) as opool,
        ):
            # small inputs first so the first chunk's compute starts early
            # SWDGE ring order: hi piece 0 (ScalarE's chain) leads, then sc
            # (V's first multiply), then the remaining nibble pieces; the ring
            # drains strictly in issue order while HWDGE gets starved
            sc_sb = cpool.tile([128, KT * BB], f16)
            hi0_sb = wqpool.tile([128, 4 * 512], u8, tag="hi_u8_0", name="hi_u8_0")
            nc.sync.dma_start(out=hi0_sb[:], in_=hi_d[:, 0 : 4 * 512])
            xt_sb = cpool.tile([128, KT * B], f16)
            nc.sync.dma_start(out=xt_sb[:], in_=xt_d[:])
            sz_sb = cpool.tile([128, KT * BB], f16)
            nc.sync.dma_start(out=sz_sb[:], in_=sz_d[:])
            bias_sb = cpool.tile([1, 1024], f32)
            nc.sync.dma_start(out=bias_sb[:], in_=bs_d[:])
            ones_sb = cpool.tile([1, B], f32)
            nc.vector.memset(ones_sb[:], 1.0)

            # nibble staging: ALL nibble traffic on the SWDGE ring so pieces
            # drain in issue order (HWDGE+SWDGE round-robin starves whichever
            # class has less ring work -- measured hi landing at 40us in v4).
            # Small first pieces shrink pipeline fill; finer interleave keeps
            # ScalarE (hi) and Vector (lo) fed evenly.
            hi_pcs = []
            lo_pcs = []
            k0 = 0
            for i, pk in enumerate(PIECES):
                if i == 0:
                    th = hi0_sb
                else:
                    th = wqpool.tile(
                        [128, pk * 512], u8, tag=f"hi_u8_{i}", name=f"hi_u8_{i}"
                    )
                tl = wqpool.tile(
                    [128, pk * 512], f16, tag=f"lo_f16_{i}", name=f"lo_f16_{i}"
                )
                hi_pcs.append((k0, k0 + pk, th))
                lo_pcs.append((k0, k0 + pk, tl))
                k0 += pk

            def _csl(i):
                p0, p1, _ = hi_pcs[i]
                return slice(p0 * 512, p1 * 512)

            nc.gpsimd.dma_start(out=sc_sb[:], in_=sc_d[:])
            nc.gpsimd.dma_start(out=lo_pcs[0][2][:], in_=lo_d[:, _csl(0)])
            for i in range(1, len(PIECES)):
                nc.gpsimd.dma_start(out=hi_pcs[i][2][:], in_=hi_d[:, _csl(i)])
                nc.gpsimd.dma_start(out=lo_pcs[i][2][:], in_=lo_d[:, _csl(i)])

            def piece_ap(pcs, k, ck):
                # return the 4D [128, ck, 32, BB] view for k-tiles [k, k+ck)
                for p0, p1, t in pcs:
                    if p0 <= k and k + ck <= p1:
                        return t[:, (k - p0) * 512 : (k - p0 + ck) * 512].rearrange(
                            "p (k r b) -> p k r b", k=ck, b=BB
                        )
                raise AssertionError("chunk crosses piece boundary")

            psU = pspool.tile([128, 512], f32)   # [0:64) hi | [64:128) lo
            psC = pspool.tile([B, BB], f32)      # -(x @ scale*zero) term

            def sc_bc(k, ck):
                return (
                    sc_sb[:, k * BB : (k + ck) * BB]
                    .rearrange("p (k b) -> p k b", b=BB)
                    .unsqueeze(2)
                    .broadcast_to((128, ck, 32, BB))
                )

            # bias enters PSUM first (K=1 ones x bias opens the groups), so
            # the last chunk's matmuls close them and the tail stays short
            nc.tensor.matmul(
                psU[0:64, :], ones_sb[:], bias_sb[:, 0:512],
                start=True, stop=False, tile_position=(0, 0),
            )
            nc.tensor.matmul(
                psU[64:128, :], ones_sb[:], bias_sb[:, 512:1024],
                start=True, stop=False, tile_position=(0, 64),
            )

            kc = 0
            for c, ck in enumerate(CHUNKS):
                # hi: ScalarE cast u8 -> bf16, then V scale-mult
                hf = nibpool.tile([128, ck * 512], f16, tag="hi_f")
                nc.scalar.activation(
                    out=hf[:].rearrange("p (k r b) -> p k r b", k=ck, b=BB),
                    in_=piece_ap(hi_pcs, kc, ck),
                    func=mybir.ActivationFunctionType.Copy, scale=1.0,
                )
                wh = wspool.tile([128, ck * 512], f16, tag="ws_hi")
                nc.vector.tensor_tensor(
                    out=wh[:].rearrange("p (k r b) -> p k r b", k=ck, b=BB),
                    in0=hf[:].rearrange("p (k r b) -> p k r b", k=ck, b=BB),
                    in1=sc_bc(kc, ck),
                    op=AluOpType.mult,
                )
                # lo: already bf16 from cast-DMA, V scale-mult
                wl = wspool.tile([128, ck * 512], f16, tag="ws_lo")
                nc.vector.tensor_tensor(
                    out=wl[:].rearrange("p (k r b) -> p k r b", k=ck, b=BB),
                    in0=piece_ap(lo_pcs, kc, ck),
                    in1=sc_bc(kc, ck),
                    op=AluOpType.mult,
                )

                for kl in range(ck):
                    k = kc + kl
                    last = k == KT - 1
                    lhsT = xt_sb[:, k * B : (k + 1) * B]
                    nc.tensor.matmul(
                        psU[0:64, :], lhsT, wh[:, kl * 512 : (kl + 1) * 512],
                        start=False, stop=last, tile_position=(0, 0),
                    )
                    nc.tensor.matmul(
                        psU[64:128, :], lhsT, wl[:, kl * 512 : (kl + 1) * 512],
                        start=False, stop=last, tile_position=(0, 64),
                    )
                    nc.tensor.matmul(
                        psC[:], lhsT, sz_sb[:, k * BB : (k + 1) * BB],
                        start=(k == 0), stop=last, tile_position=(0, 0),
                    )
                kc += ck

            # epilogue: y_half = psU_half + (-x @ sc*zero) broadcast over r
            psC_sb = opool.tile([B, BB], f32)
            nc.scalar.copy(out=psC_sb[:], in_=psC[:])
            psC_bc = psC_sb[:].unsqueeze(1).broadcast_to((B, 32, BB))
            out_sb = opool.tile([B, 1024], f32)
            nc.vector.scalar_tensor_tensor(
                out=out_sb[:, 0:512].rearrange("t (r b) -> t r b", b=BB),
                in0=psU[0:64, :].rearrange("t (r b) -> t r b", b=BB),
                scalar=1.0,
                in1=psC_bc,
                op0=AluOpType.mult,
                op1=AluOpType.add,
            )
            nc.vector.scalar_tensor_tensor(
                out=out_sb[:, 512:1024].rearrange("t (r b) -> t r b", b=BB),
                in0=psU[64:128, :].rearrange("t (r b) -> t r b", b=BB),
                scalar=1.0,
                in1=psC_bc,
                op0=AluOpType.mult,
                op1=AluOpType.add,
            )
            nc.sync.dma_start(out=y_d[:], in_=out_sb[:])

    nc.compile()
    return nc


def _get_nc():
    if "nc" not in _CACHE:
        _CACHE["nc"] = _build_nc()
    return _CACHE["nc"]


def _prep_inputs(x, W_q, scale, zero, bias):
    """Host-side shard + layout prep (dtype narrowing / bit repack / transposes)."""
    xt = (
        x.T.reshape(KT, 128, B).transpose(1, 0, 2).reshape(128, KT * B)
    ).astype(ml_dtypes.bfloat16)  # (p, (k t))
    wq_u8 = W_q.astype(np.uint8)
    hi_u8 = (wq_u8 >> 4).astype(np.uint8)
    lo_u8 = (wq_u8 & 0xF).astype(np.uint8)
    # negated so the PSUM term is directly addable: psC = -x @ (scale*zero)
    sz_full = -(scale.astype(np.float64) * zero.astype(np.float64)).astype(np.float32)

    def wlayout(arr_m):
        # arr_m: (32, NGC) one core's nibble slice -> [p, (k, r, b)]
        a = arr_m.reshape(32, BB, IN)          # (r, b, in)
        a = a.transpose(2, 0, 1)               # (in, r, b): col = r*16+b
        a = a.reshape(KT, 128, 512)            # (k, p, rb)
        a = a.transpose(1, 0, 2)               # (p, k, rb)
        return np.ascontiguousarray(a.reshape(128, KT * 512))

    in_maps = []
    for m in range(NCORES):
        sl = slice(m * NGC, (m + 1) * NGC)
        sc_m = (
            scale[0, sl]
            .reshape(BB, IN)
            .T.reshape(KT, 128, BB)
            .transpose(1, 0, 2)
            .reshape(128, KT * BB)
        ).astype(ml_dtypes.bfloat16)
        sz_m = (
            sz_full[0, sl]
            .reshape(BB, IN)
            .T.reshape(KT, 128, BB)
            .transpose(1, 0, 2)
            .reshape(128, KT * BB)
        ).astype(ml_dtypes.bfloat16)
        # out col c = h*512 + r*16 + b  <->  global out o = (h*32+r)*128 + m*16 + b
        bs_m = (
            bias.reshape(GS, 128)[:, m * BB : (m + 1) * BB]  # (gs, b)
            .reshape(1, 1024)
            .astype(np.float32)
        )
        in_maps.append(
            {
                "xt": xt,
                "hi": wlayout(hi_u8[:, sl]),
                "lo": wlayout(lo_u8[:, sl]),
                "sc": np.ascontiguousarray(sc_m),
                "sz": np.ascontiguousarray(sz_m),
                "bs": bs_m,
            }
        )
    return in_maps


def _gather(results):
    ybig = np.stack([results[m]["y"] for m in range(NCORES)], axis=1)  # (t, m, 1024)
    ybig = ybig.reshape(B, NCORES, 2, 32, BB)  # (t, m, h, r, b)
    return np.ascontiguousarray(
        ybig.transpose(0, 2, 3, 1, 4).reshape(B, OUT)
    )  # o = (h*32+r)*128 + m*16 + b


def run_on_hw(x, W_q, scale, zero, bias, trace=False, **trace_kw):
    """Returns (y_full, BassKernelResults)."""
    from concourse.bass_utils import run_bass_kernel_spmd

    nc = _get_nc()
    in_maps = _prep_inputs(x, W_q, scale, zero, bias)
    res = run_bass_kernel_spmd(
        nc, in_maps, list(range(NCORES)), trace=trace, **trace_kw
    )
    return _gather(res.results), res


def kernel(x, W_q, scale, zero, bias):
    y, _ = run_on_hw(x, W_q, scale, zero, bias, trace=False)
    return y
